# revision 1
# baseline (speedup 1.0000x reference)
"""LocalFrameAttentionWithDiffuser on 8 TRN2 NeuronCores.

Sharding: head-parallel. Each core computes 2 of the 16 heads end-to-end
(QKV projection for its 128 hd-dims, chunked local attention, partial
output projection Y_c = O_c @ Wo[c-slice]); the host sums the 8 partial
Y tensors and adds the bias.

Shapes (hardcoded from the problem):
  x [1,16,256,1024] -> tokens T=4096, D=1024, H=16 heads, HD=64,
  chunks C=4 of L=1024 tokens; chunk i attends to chunks {i-1, i}
  (chunk 0 only to itself).

Device pipeline (per core):
  - All three projections run as fp8e4 DoubleRow matmuls (contraction
    2x128 per instruction, 0.5 cycles/row) with residual compensation:
    X ~= X8 + X8lo and W ~= W8 + W8lo (all host-quantized fp8, same
    scale family), and the PSUM accumulates X8@W8 + X8@W8lo + X8lo@W8.
    This is 0.75x the PE cost of bf16 at bf16-class accuracy (~0.2%);
    plain fp8 is NOT usable anywhere in the data path because
    multiplicative noise on scores/A/V lands ~1:1 in the output (the
    output is a random-sign sum, so noise does not average out).
  - Wq/Wk are pre-scaled by 64 and Wv by 32 so the fp8 weights sit in
    the normal range; the Q/K scale is compensated in the exp's scale
    argument, and the V scale cancels in the softmax normalize (the
    denominator ones-column is also 32).
  - V is produced directly in [tok, hd] layout (X8 tiles stationary,
    Wv8 moving) - no transpose needed.
  - Scores S^T = K^T.T @ Q^T per (chunk, head) with ctx on partitions;
    two 128-ctx tiles share one 2-bank PSUM tile so a single Exp
    activation covers 1024 elements (halves Act-engine overhead).
  - AV uses A-tiles as stationary (bf16) and V [128, 65] as moving
    (64 hd dims + the 32s column -> softmax denominators for free),
    producing O as [tok, hd]; normalization is a reciprocal +
    per-partition tensor_scalar_mul, then one bf16 PE transpose per
    128-token block back to [hd, tok] for the output projection.
  - Emission is software-pipelined: AV/normalize/out-proj of the
    previous (chunk, tok-half) and the next chunk's projections are
    interleaved between score/exp pairs so the Activation engine
    (the critical resource at ~116us busy) rarely starves.
"""

from contextlib import ExitStack

import numpy as np
import ml_dtypes

import concourse.bass as bass
import concourse.tile as tile
from concourse import bacc, mybir
from concourse.bass_utils import run_bass_kernel_spmd

F32 = mybir.dt.float32
BF16 = mybir.dt.bfloat16
FP8 = mybir.dt.float8e4

B, F, N, D = 1, 16, 256, 1024
H, HD = 16, 64
CS = 4
C = F // CS            # 4 chunks
L = CS * N             # 1024 tokens per chunk
T = F * N              # 4096 tokens
NCORES = 8
HPC = H // NCORES      # 2 heads per core
HDB = HPC * HD         # 128 hd dims per core
QKSCALE = 64.0         # fp8 pre-scale for Wq/Wk
VSCALE = 32.0          # fp8 pre-scale for Wv (cancels in normalize)
EXP_SCALE = (1.0 / np.sqrt(HD)) / (QKSCALE * QKSCALE)

NDR = D // 256         # 4 DoubleRow contraction tiles
NTB = T // 128         # 32 128-token blocks
NPAIR = NTB // 2       # 16 ctx-tile pairs
NW = 6 * NDR           # weight blocks: (q,k,v) x (hi,lo) x NDR


def build_kernel(nc, tc, outs, ins, ctx):
    y = outs["y"]

    sb = ctx.enter_context(tc.tile_pool(name="sb", bufs=1))
    a_pool = ctx.enter_context(tc.tile_pool(name="attn", bufs=26))
    small = ctx.enter_context(tc.tile_pool(name="small", bufs=8))
    otp = ctx.enter_context(tc.tile_pool(name="otp", bufs=6))
    ysb_pool = ctx.enter_context(tc.tile_pool(name="ysb", bufs=4))
    ps = ctx.enter_context(tc.tile_pool(name="ps", bufs=1, space="PSUM"))

    # ---- persistent SBUF tensors ----
    x8 = [sb.tile([128, 2, T], FP8, name=f"x8_{r}") for r in range(NDR)]
    x8lo = [sb.tile([128, 2, T], FP8, name=f"x8lo_{r}") for r in range(NDR)]
    # packed weights: [(q,k,v) x (hi,lo)] x NDR blocks of [128, 2, 128]
    wpk = sb.tile([128, NW, 2, HDB], FP8)

    def wblk(proj, lo, r):
        return wpk[:, (proj * 2 + lo) * NDR + r, :, :]

    wo_sb = sb.tile([128, D], BF16)
    id_sb = sb.tile([128, 128], BF16)
    qt_sb = sb.tile([128, T], BF16)     # Q^T x 64 (2 heads stacked)
    kt_sb = sb.tile([128, T], BF16)     # K^T x 64
    # V x 32 per (head, ctx-tile-pair): [128 tok, 2*(64 hd + 32s col)] bf16
    v_sb = [[sb.tile([128, 2 * (HD + 1)], BF16, name=f"v{h}_{p}")
             for p in range(NPAIR)] for h in range(HPC)]

    # warm up the Act engine's exp table before real data arrives
    warm = small.tile([128, 8], F32, tag="warm")
    nc.vector.memset(warm[:], 0.0)
    nc.scalar.activation(warm[:], warm[:], mybir.ActivationFunctionType.Exp,
                         scale=1.0)

    # ---- input DMAs (SP queue), ordered for pipeline startup ----
    nc.sync.dma_start(wpk[:], ins["wpk"][:])
    for r in range(NDR):
        nc.sync.dma_start(x8[r][:, :, 0:1024], ins[f"x8_{r}"][:, :, 0:1024])
    for r in range(NDR):
        nc.sync.dma_start(x8lo[r][:, :, 0:1024], ins[f"x8lo_{r}"][:, :, 0:1024])
    nc.sync.dma_start(id_sb[:], ins["ident"][:])
    for r in range(NDR):
        nc.sync.dma_start(x8[r][:, :, 1024:2048], ins[f"x8_{r}"][:, :, 1024:2048])
        nc.sync.dma_start(x8lo[r][:, :, 1024:2048],
                          ins[f"x8lo_{r}"][:, :, 1024:2048])
    nc.sync.dma_start(wo_sb[:], ins["wo"][:])
    for r in range(NDR):
        nc.sync.dma_start(x8[r][:, :, 2048:T], ins[f"x8_{r}"][:, :, 2048:T])
        nc.sync.dma_start(x8lo[r][:, :, 2048:T], ins[f"x8lo_{r}"][:, :, 2048:T])

    # 32s columns of every V tile (cols 0:64/65:129 overwritten later)
    for h in range(HPC):
        for p in range(NPAIR):
            nc.gpsimd.memset(v_sb[h][p][:], VSCALE)

    DR = mybir.MatmulPerfMode.DoubleRow

    def proj_qk(j, proj, dst, nm):
        """3-term fp8 DoubleRow projection of one 512-token tile (Q or K)."""
        def emit():
            tok = bass.ts(j, 512)
            pp = ps.tile([128, 512], F32, tag="bk", bufs=4, name=f"p{nm}{j}")
            mms = ([(wblk(proj, 0, r), x8[r]) for r in range(NDR)]
                   + [(wblk(proj, 1, r), x8[r]) for r in range(NDR)]
                   + [(wblk(proj, 0, r), x8lo[r]) for r in range(NDR)])
            for i, (w, xx) in enumerate(mms):
                nc.tensor.matmul(pp[:], w, xx[:, :, tok],
                                 start=(i == 0), stop=(i == len(mms) - 1),
                                 perf_mode=DR)
            nc.vector.tensor_copy(dst[:, tok], pp[:])
        return emit

    def proj_v(tb):
        """3-term fp8 DoubleRow V projection of one 128-token block,
        directly in [tok, hd] layout -> v_sb pair halves (bf16)."""
        def emit():
            tok = bass.ts(tb, 128)
            pv = ps.tile([128, HDB], F32, tag="bk", bufs=4, name=f"pv{tb}")
            mms = ([(x8[r], wblk(2, 0, r)) for r in range(NDR)]
                   + [(x8[r], wblk(2, 1, r)) for r in range(NDR)]
                   + [(x8lo[r], wblk(2, 0, r)) for r in range(NDR)])
            for i, (xx, w) in enumerate(mms):
                nc.tensor.matmul(pv[:], xx[:, :, tok], w,
                                 start=(i == 0), stop=(i == len(mms) - 1),
                                 perf_mode=DR)
            for h in range(HPC):
                nc.vector.tensor_copy(
                    v_sb[h][tb // 2][:, (tb % 2) * (HD + 1):(tb % 2) * (HD + 1) + HD],
                    pv[:, h * HD:(h + 1) * HD])
        return emit

    a_tiles = {}
    on_tiles = {}

    def av_head(c, th, h, tb, cts):
        """AV + normalize for one head of one 128-token block; both heads
        write the same [128, 128] normalized-O tile (cols h*64..)."""
        def emit():
            tokblk = c * 8 + th * 4 + tb
            o2 = ps.tile([128, HD + 1], F32, tag="bk", bufs=4,
                         name=f"o2_{tokblk}_{h}")
            for ci, ct in enumerate(cts):
                half = ct % 2
                nc.tensor.matmul(
                    o2[:],
                    a_tiles[(c, th, h, ct // 2)][:, half * 512 + tb * 128:
                                                 half * 512 + tb * 128 + 128],
                    v_sb[h][ct // 2][:, half * (HD + 1):half * (HD + 1) + HD + 1],
                    start=(ci == 0), stop=(ci == len(cts) - 1))
            rec = small.tile([128, 1], F32, tag="rec", name=f"rc{tokblk}_{h}")
            nc.vector.reciprocal(rec[:], o2[:, HD:HD + 1])
            if h == 0:
                on_tiles[tokblk] = otp.tile([128, 2 * HD], BF16, tag="on",
                                            bufs=10, name=f"on{tokblk}")
            nc.vector.tensor_scalar_mul(on_tiles[tokblk][:, h * HD:(h + 1) * HD],
                                        o2[:, 0:HD], rec[:])
        return emit

    def finish_block(c, th, tb):
        """Transpose normalized O back to [hd, tok], out-project, DMA out."""
        def emit():
            tokblk = c * 8 + th * 4 + tb
            ot_ps = ps.tile([128, 128], BF16, tag="bk", bufs=4, name=f"otp{tokblk}")
            nc.tensor.transpose(ot_ps[:], on_tiles[tokblk][:], id_sb[:])
            ot = otp.tile([128, 128], BF16, tag="ot", name=f"ot{tokblk}")
            nc.vector.tensor_copy(ot[:], ot_ps[:])
            ysb = ysb_pool.tile([128, D], BF16, tag="y", name=f"ysb{tokblk}")
            for dh in range(2):
                yp = ps.tile([128, 512], F32, tag="bk", bufs=4,
                             name=f"yp{tokblk}_{dh}")
                nc.tensor.matmul(yp[:], ot[:], wo_sb[:, bass.ts(dh, 512)],
                                 start=True, stop=True)
                nc.vector.tensor_copy(ysb[:, bass.ts(dh, 512)], yp[:])
            nc.sync.dma_start(y[bass.ts(tokblk, 128), :], ysb[:])
        return emit

    # ---- software-pipelined main loop ----
    filler = []

    def drain(k):
        nonlocal filler
        for f in filler[:k]:
            f()
        filler = filler[k:]

    # prologue: chunk 0 Q/K projections inline; V via filler
    for j in (0, 1):
        proj_qk(j, 1, kt_sb, "k")()
        proj_qk(j, 0, qt_sb, "q")()
    filler.extend(proj_v(tb) for tb in range(8))

    blocks = [(c, th, h) for c in range(C) for th in range(2) for h in range(HPC)]
    for bi, (c, th, h) in enumerate(blocks):
        cts = list(range(max(0, 8 * (c - 1)), 8 * (c + 1)))
        pairs = sorted({ct // 2 for ct in cts})
        tok = bass.ds(c * L + th * 512, 512)

        # enqueue deferred work: next chunk's projections, split across the
        # first two blocks of this chunk so X DMAs have time to land
        if th == 0 and h == 0 and c + 1 < C:
            for j in (2 * (c + 1), 2 * (c + 1) + 1):
                filler.append(proj_qk(j, 1, kt_sb, "k"))
                filler.append(proj_qk(j, 0, qt_sb, "q"))
            filler.extend(proj_v(tb) for tb in range(8 * (c + 1), 8 * (c + 1) + 4))
        if th == 0 and h == 1 and c + 1 < C:
            filler.extend(proj_v(tb) for tb in range(8 * (c + 1) + 4, 8 * (c + 2)))

        # drain rate: 1/pair in steady state, more when the end nears
        pairs_left = sum(
            len(range(max(0, 8 * (cc - 1)), 8 * (cc + 1))) // 2
            for (cc, _, _) in blocks[bi:])
        per = max(1, -(-(len(filler) + 8) // max(1, pairs_left)))
        for p in pairs:
            sc = ps.tile([128, 1024], F32, tag="sc", bufs=2, name=f"sc{bi}_{p}")
            for half in range(2):
                ct = 2 * p + half
                nc.tensor.matmul(
                    sc[:, bass.ts(half, 512)],
                    kt_sb[h * HD:(h + 1) * HD, bass.ts(ct, 128)],
                    qt_sb[h * HD:(h + 1) * HD, tok],
                    start=True, stop=True)
            at = a_pool.tile([128, 1024], BF16, tag="a", name=f"a{bi}_{p}")
            nc.scalar.activation(at[:], sc[:],
                                 mybir.ActivationFunctionType.Exp,
                                 scale=EXP_SCALE)
            a_tiles[(c, th, h, p)] = at
            drain(per)

        filler.extend(av_head(c, th, h, tb, cts) for tb in range(4))
        if h == HPC - 1:
            filler.extend(finish_block(c, th, tb) for tb in range(4))

    drain(len(filler))


_CACHE = {}


def _build():
    if "nc" in _CACHE:
        return _CACHE["nc"]
    nc = bacc.Bacc(
        "TRN2",
        target_bir_lowering=False,
        debug=False,
        enable_asserts=False,
        num_devices=NCORES,
    )
    ins = {
        "wpk": nc.dram_tensor("wpk", [128, NW, 2, HDB], FP8,
                              kind="ExternalInput").ap(),
        "wo": nc.dram_tensor("wo", [HDB, D], BF16, kind="ExternalInput").ap(),
        "ident": nc.dram_tensor("ident", [128, 128], BF16,
                                kind="ExternalInput").ap(),
    }
    for r in range(NDR):
        ins[f"x8_{r}"] = nc.dram_tensor(f"x8_{r}", [128, 2, T], FP8,
                                        kind="ExternalInput").ap()
        ins[f"x8lo_{r}"] = nc.dram_tensor(f"x8lo_{r}", [128, 2, T], FP8,
                                          kind="ExternalInput").ap()
    outs = {"y": nc.dram_tensor("y", [T, D], BF16, kind="ExternalOutput").ap()}
    with tile.TileContext(nc, trace_sim=False) as tc:
        with ExitStack() as kctx:
            build_kernel(nc, tc, outs, ins, kctx)
    nc.compile()
    _CACHE["nc"] = nc
    return nc


def make_in_maps(x, Wq, Wk, Wv, Wo, bo):
    f8 = ml_dtypes.float8_e4m3
    xt32 = np.asarray(x, dtype=np.float32).reshape(T, D).T  # [D, T]
    # DoubleRow layout [r][p, i, t] = X^T[r*256 + i*128 + p, t], hi + residual
    xdr = np.ascontiguousarray(xt32.reshape(NDR, 2, 128, T).transpose(0, 2, 1, 3))
    x8 = xdr.astype(f8)
    x8lo = (xdr - x8.astype(np.float32)).astype(f8)
    ident = np.eye(128, dtype=np.float32).astype(ml_dtypes.bfloat16)

    def wdr(w, hs, scale):
        """[D, 128] weight slice -> DR layout [NDR, 128, 2, 128] hi + lo."""
        m = np.asarray(w, np.float32)[:, hs] * scale
        m = np.ascontiguousarray(m.reshape(NDR, 2, 128, HDB).transpose(0, 2, 1, 3))
        hi = m.astype(f8)
        lo = (m - hi.astype(np.float32)).astype(f8)
        return hi, lo

    in_maps = []
    for core in range(NCORES):
        hs = slice(core * HDB, (core + 1) * HDB)
        wo = np.ascontiguousarray(
            np.asarray(Wo, np.float32)[hs, :]).astype(ml_dtypes.bfloat16)
        # packed weight tensor: [(q,k,v) x (hi,lo)] x NDR of [128, 2, 128]
        blocks = []
        for w, scale in ((Wq, QKSCALE), (Wk, QKSCALE), (Wv, VSCALE)):
            hi, lo = wdr(w, hs, scale)
            blocks.append(hi)
            blocks.append(lo)
        wpk = np.ascontiguousarray(
            np.stack(blocks, axis=0)            # [6, NDR, 128, 2, 128]
            .transpose(2, 0, 1, 3, 4)           # [128, 6, NDR, 2, 128]
            .reshape(128, NW, 2, HDB))
        im = {"wpk": wpk, "wo": wo, "ident": ident}
        for r in range(NDR):
            im[f"x8_{r}"] = x8[r]
            im[f"x8lo_{r}"] = x8lo[r]
        in_maps.append(im)
    return in_maps


def kernel(x, Wq, Wk, Wv, Wo, bo, _trace=False, _tmpdir=None):
    nc = _build()
    in_maps = make_in_maps(x, Wq, Wk, Wv, Wo, bo)
    res = run_bass_kernel_spmd(
        nc, in_maps, core_ids=list(range(NCORES)),
        trace=_trace, tmpdir=_tmpdir,
        **({"trace_cores": list(range(NCORES))} if _trace else {}),
    )
    if _trace:
        kernel.last_results = res
    y = np.zeros((T, D), dtype=np.float32)
    for r in res.results:
        y += np.asarray(r["y"], dtype=np.float32)
    y += np.asarray(bo, dtype=np.float32).reshape(1, D)
    return y.reshape(B, F, N, D)



# revision 4
# speedup vs baseline: 1.0285x; 1.0285x over previous
"""LocalFrameAttentionWithDiffuser on 8 TRN2 NeuronCores.

Sharding: head-parallel. Each core computes 2 of the 16 heads end-to-end
(QKV projection for its 128 hd-dims, chunked local attention, partial
output projection Y_c = O_c @ Wo[c-slice]); the host sums the 8 partial
Y tensors and adds the bias.

Shapes (hardcoded from the problem):
  x [1,16,256,1024] -> tokens T=4096, D=1024, H=16 heads, HD=64,
  chunks C=4 of L=1024 tokens; chunk i attends to chunks {i-1, i}
  (chunk 0 only to itself).

Device pipeline (per core):
  - All three projections run as fp8e4 DoubleRow matmuls with residual
    compensation (X ~= X8 + X8lo, W ~= W8 + W8lo; PSUM accumulates
    X8@W8 + X8@W8lo + X8lo@W8): 0.75x the PE cost of bf16 at
    bf16-class accuracy.
  - Scores S^T = K^T.T @ Q^T per (chunk, head) with ctx on partitions;
    two 128-ctx tiles share one 2-bank PSUM tile so a single Exp
    activation covers 1024 elements (halves Act-engine overhead).
  - AV uses A-tiles as stationary (bf16) and V [128, 65] as moving
    (64 hd dims + a 32s column -> softmax denominators for free).
  - All PSUM->SBUF drains run on DVE (GPSIMD cannot access PSUM on
    TRN2); Pool handles only the SBUF memsets. Producer-side copies
    (Q^T/K^T, V) are emitted ahead of consumer-side ones (O^T, Y).
  - Emission is a cycle-budgeted software pipeline: between each score
    quad (4 matmuls, ~850ns PE) the scheduler drains ~1250ns of
    deferred PE microtasks (next chunk's projections, AV, transpose,
    out-proj) so the PE never idles and stays at its full p-state
    clock, while the Act engine's exp stream runs ~90% occupancy.
"""

from collections import deque
from contextlib import ExitStack

import numpy as np
import ml_dtypes

import concourse.bass as bass
import concourse.tile as tile
from concourse import bacc, mybir
from concourse.bass_utils import run_bass_kernel_spmd

F32 = mybir.dt.float32
BF16 = mybir.dt.bfloat16
FP8 = mybir.dt.float8e4

B, F, N, D = 1, 16, 256, 1024
H, HD = 16, 64
CS = 4
C = F // CS            # 4 chunks
L = CS * N             # 1024 tokens per chunk
T = F * N              # 4096 tokens
NCORES = 8
HPC = H // NCORES      # 2 heads per core
HDB = HPC * HD         # 128 hd dims per core
QKSCALE = 64.0         # fp8 pre-scale for Wq/Wk
VSCALE = 32.0          # fp8 pre-scale for Wv (cancels in normalize)
EXP_SCALE = (1.0 / np.sqrt(HD)) / (QKSCALE * QKSCALE)

NDR = D // 256         # 4 DoubleRow contraction tiles
NTB = T // 128         # 32 128-token blocks
NPAIR = NTB // 2       # 16 ctx-tile pairs
NW = 6 * NDR           # weight blocks: (q,k,v) x (hi,lo) x NDR

DR = mybir.MatmulPerfMode.DoubleRow


def build_kernel(nc, tc, outs, ins, ctx):
    y = outs["y"]

    sb = ctx.enter_context(tc.tile_pool(name="sb", bufs=1))
    a_pool = ctx.enter_context(tc.tile_pool(name="attn", bufs=34))
    small = ctx.enter_context(tc.tile_pool(name="small", bufs=8))
    otp = ctx.enter_context(tc.tile_pool(name="otp", bufs=6))
    ysb_pool = ctx.enter_context(tc.tile_pool(name="ysb", bufs=4))
    ps = ctx.enter_context(tc.tile_pool(name="ps", bufs=1, space="PSUM"))

    # ---- persistent SBUF tensors ----
    # x in DoubleRow layout, all NDR contraction tiles in one tensor so a
    # token range is ONE DMA transfer (the HWDGE issues serialize at
    # ~625ns each; fewer, larger transfers keep the serial pipe full)
    x8 = sb.tile([128, NDR, 2, T], FP8)
    x8lo = sb.tile([128, NDR, 2, T], FP8)
    wpk = sb.tile([128, NW, 2, HDB], FP8)

    # host packing order: [K-hi, Q-hi, K-lo, Q-lo, V-hi, V-lo] x NDR, so the
    # first DMA piece [0:2*NDR] is exactly the hi weights of K and Q
    WIDX = {(1, 0): 0, (0, 0): 1, (1, 1): 2, (0, 1): 3, (2, 0): 4, (2, 1): 5}

    def wblk(proj, lo, r):
        return wpk[:, WIDX[(proj, lo)] * NDR + r, :, :]

    wo_sb = sb.tile([128, D], BF16)
    id_sb = sb.tile([128, 128], BF16)
    qt_sb = sb.tile([128, T], BF16)     # Q^T x 64 (2 heads stacked)
    kt_sb = sb.tile([128, T], BF16)     # K^T x 64
    # V x 32 per (head, ctx-tile-pair): [128 tok, 2*(64 hd + 32s col)] bf16
    v_sb = [[sb.tile([128, 2 * (HD + 1)], BF16, name=f"v{h}_{p}")
             for p in range(NPAIR)] for h in range(HPC)]

    # warm up the Act engine's exp table before real data arrives
    warm = small.tile([128, 8], F32, tag="warm")
    nc.vector.memset(warm[:], 0.0)
    nc.scalar.activation(warm[:], warm[:], mybir.ActivationFunctionType.Exp,
                         scale=1.0)

    # warm up the PE p-state: a stream of throwaway matmuls keeps the PE
    # busy-streak alive from ~1us, so the first projections run at the
    # full 2.4GHz clock instead of the 1.2GHz mid p-state
    wmm = small.tile([128, 512], BF16, tag="wmm")
    nc.vector.memset(wmm[:], 0.0)
    wps = ps.tile([128, 512], F32, tag="sc", bufs=2, name="warm_ps")
    for _ in range(8):
        nc.tensor.matmul(wps[:], wmm[:, 0:128], wmm[:], start=True, stop=True)
    nc.vector.tensor_copy(wmm[:, 0:8], wps[:, 0:8])

    # ---- input DMAs ----
    # The cost model's DMA data path is a single serial pipe served in issue
    # order, so the head sequence is strictly FIFO on the SP queue: K/Q hi
    # weights, then the chunk-0 head blob (hi pieces, lo pieces), then K/Q lo
    # weights interleaved. The Act queue carries NO input DMAs (each would
    # cost its sequencer 667ns, delaying the exp stream).
    nc.sync.dma_start(wpk[:, 0:2 * NDR], ins["wpk"][:, 0:2 * NDR])
    nc.sync.dma_start(x8[:, :, :, 0:512], ins["x8"][:, :, :, 0:512])
    nc.sync.dma_start(wpk[:, 2 * NDR:4 * NDR], ins["wpk"][:, 2 * NDR:4 * NDR])
    nc.sync.dma_start(x8lo[:, :, :, 0:512], ins["x8lo"][:, :, :, 0:512])
    # x for token tile j=1 (needed by fillers inside chunk 0)
    nc.sync.dma_start(x8[:, :, :, 512:1024], ins["x8"][:, :, :, 512:1024])
    nc.sync.dma_start(wpk[:, 4 * NDR:6 * NDR], ins["wpk"][:, 4 * NDR:6 * NDR])
    nc.sync.dma_start(x8lo[:, :, :, 512:1024], ins["x8lo"][:, :, :, 512:1024])
    nc.sync.dma_start(id_sb[:], ins["ident"][:])
    nc.sync.dma_start(wo_sb[:], ins["wo"][:])
    # the rest of x, in arrival order of use
    nc.sync.dma_start(x8[:, :, :, 1024:2048], ins["x8"][:, :, :, 1024:2048])
    nc.sync.dma_start(x8lo[:, :, :, 1024:2048], ins["x8lo"][:, :, :, 1024:2048])
    nc.sync.dma_start(x8[:, :, :, 2048:T], ins["x8"][:, :, :, 2048:T])
    nc.sync.dma_start(x8lo[:, :, :, 2048:T], ins["x8lo"][:, :, :, 2048:T])

    # 32s columns of every V tile (cols 0:64/65:129 overwritten later)
    for h in range(HPC):
        for p in range(NPAIR):
            nc.gpsimd.memset(v_sb[h][p][:], VSCALE)

    # ---------- microtask scheduler ----------
    # fillers: FIFO of (pe_cycles, emit_fn). Between score quads the main
    # loop drains fillers against a running PE-cycle budget.
    fillers = deque()
    filler_cycles = 0

    def push(cost, fn):
        nonlocal filler_cycles
        fillers.append((cost, fn))
        filler_cycles += cost

    def drain_cycles(budget):
        nonlocal filler_cycles
        spent = 0
        while fillers and spent < budget:
            cost, fn = fillers.popleft()
            fn()
            spent += cost
            filler_cycles -= cost
        return spent

    def drain_all():
        drain_cycles(1 << 60)

    # ---------- projections ----------
    pp_tiles = {}

    def xsrc(lo, r, t0, t1):
        """x8 operand [128, 2, t1-t0] for DoubleRow tile r."""
        return (x8lo if lo else x8)[:, r, :, t0:t1]

    def proj_qk_group(j, proj, dst, term, nm):
        """One 3-term fp8 DR projection of a 512-token tile, split into
        3 filler groups of 4 matmuls (1024 PE cycles each)."""
        def emit():
            tok = bass.ts(j, 512)
            if term == 0:
                pp_tiles[(proj, j)] = ps.tile([128, 512], F32, tag="bk",
                                              bufs=4, name=f"p{nm}{j}")
            pp = pp_tiles[(proj, j)]
            lo_x = term == 2
            wlo = 1 if term == 1 else 0
            for i in range(NDR):
                nc.tensor.matmul(pp[:], wblk(proj, wlo, i),
                                 xsrc(lo_x, i, j * 512, j * 512 + 512),
                                 start=(term == 0 and i == 0),
                                 stop=(term == 2 and i == NDR - 1),
                                 perf_mode=DR)
            if term == 2:
                if j == 0 and proj == 0:
                    # head: Q^T j0 copy on the still-idle Act engine so it
                    # overlaps the K^T copy on DVE
                    nc.scalar.copy(dst[:, tok], pp[:])
                else:
                    nc.vector.tensor_copy(dst[:, tok], pp[:])
                del pp_tiles[(proj, j)]
        return (1024, emit)

    def proj_v(tb):
        """3-term fp8 DR V projection of one 128-token block, directly in
        [tok, hd] layout -> v_sb pair halves (bf16)."""
        def emit():
            t0 = tb * 128
            pv = ps.tile([128, HDB], F32, tag="bk", bufs=4, name=f"pv{tb}")
            mms = ([(xsrc(0, r, t0, t0 + 128), wblk(2, 0, r)) for r in range(NDR)]
                   + [(xsrc(0, r, t0, t0 + 128), wblk(2, 1, r)) for r in range(NDR)]
                   + [(xsrc(1, r, t0, t0 + 128), wblk(2, 0, r)) for r in range(NDR)])
            for i, (xx, w) in enumerate(mms):
                nc.tensor.matmul(pv[:], xx, w,
                                 start=(i == 0), stop=(i == len(mms) - 1),
                                 perf_mode=DR)
            for h in range(HPC):
                nc.vector.tensor_copy(
                    v_sb[h][tb // 2][:, (tb % 2) * (HD + 1):(tb % 2) * (HD + 1) + HD],
                    pv[:, h * HD:(h + 1) * HD])
        return (768, emit)

    # ---------- attention ----------
    a_tiles = {}
    on_tiles = {}
    ot_tiles = {}

    o2_tiles = {}

    def av_head(c, th, h, tb, cts):
        """AV matmuls for one head of one 128-token block (PE only)."""
        def emit():
            tokblk = c * 8 + th * 4 + tb
            o2 = ps.tile([128, HD + 1], F32, tag="bk", bufs=4,
                         name=f"o2_{tokblk}_{h}")
            o2_tiles[(tokblk, h)] = o2
            for ci, ct in enumerate(cts):
                at = a_tiles[(c, th, h, ct // 2)]
                off = (ct % 2) * 512 + tb * 128
                nc.tensor.matmul(
                    o2[:], at[:, off:off + 128],
                    v_sb[h][ct // 2][:, (ct % 2) * (HD + 1):(ct % 2) * (HD + 1) + HD + 1],
                    start=(ci == 0), stop=(ci == len(cts) - 1))
        return (65 * len(cts), emit)

    def norm(c, th, h, tb):
        """Normalize one head of one 128-token block; the multiply goes to
        Pool for h==1 of the final block so the tail norms run in parallel.
        Both heads write the same [128, 128] normalized-O tile."""
        def emit():
            tokblk = c * 8 + th * 4 + tb
            o2 = o2_tiles.pop((tokblk, h))
            rec = small.tile([128, 1], F32, tag="rec", name=f"rc{tokblk}_{h}")
            nc.vector.reciprocal(rec[:], o2[:, HD:HD + 1])
            if h == 0:
                on_tiles[tokblk] = otp.tile([128, 2 * HD], BF16, tag="on",
                                            bufs=14, name=f"on{tokblk}")
            dst = on_tiles[tokblk][:, h * HD:(h + 1) * HD]
            if c == C - 1 and th == 1 and h == 1:
                # tail: the Act engine's exp stream is finished by now, so
                # run the normalize multiply there in parallel with DVE
                nc.scalar.activation(dst, o2[:, 0:HD],
                                     mybir.ActivationFunctionType.Copy,
                                     scale=rec[:])
            else:
                nc.vector.tensor_scalar_mul(dst, o2[:, 0:HD], rec[:])
        return (16, emit)

    def fin1(tokblk):
        """Transpose normalized O back to [hd, tok] (PE) + copy; tail
        blocks copy on the then-idle Act engine instead of DVE."""
        def emit():
            ot_ps = ps.tile([128, 128], BF16, tag="bk", bufs=4,
                            name=f"otp{tokblk}")
            nc.tensor.transpose(ot_ps[:], on_tiles[tokblk][:], id_sb[:])
            ot = otp.tile([128, 128], BF16, tag="ot", name=f"ot{tokblk}")
            if tokblk >= 28:
                nc.scalar.copy(ot[:], ot_ps[:])
            else:
                nc.vector.tensor_copy(ot[:], ot_ps[:])
            ot_tiles[tokblk] = ot
        return (128, emit)

    def fin2(tokblk):
        """Out-projection of one 128-token block + DVE/Pool copies + DMA out.
        Each yp half is drained by two parallel 256-col copies (DVE + Pool)
        so the PSUM ring slot frees fast; the y DMA alternates between the
        SP and Act HWDGE queues to halve issue serialization."""
        def emit():
            ysb = ysb_pool.tile([128, D], BF16, tag="y", name=f"ysb{tokblk}")
            for dh in range(2):
                yp = ps.tile([128, 512], F32, tag="bk", bufs=4,
                             name=f"yp{tokblk}_{dh}")
                nc.tensor.matmul(yp[:], ot_tiles[tokblk][:],
                                 wo_sb[:, bass.ts(dh, 512)],
                                 start=True, stop=True)
                if tokblk in (28, 29) and dh == 1:
                    nc.scalar.copy(ysb[:, bass.ts(dh, 512)], yp[:])
                else:
                    nc.vector.tensor_copy(ysb[:, bass.ts(dh, 512)], yp[:])
                if tokblk >= 30:
                    # last two blocks: DMA each half as soon as it lands,
                    # on alternating queues (Act's exp stream is done)
                    eng = nc.sync if dh == 0 else nc.scalar
                    eng.dma_start(y[bass.ts(tokblk, 128), bass.ts(dh, 512)],
                                  ysb[:, bass.ts(dh, 512)])
            if tokblk < 30:
                eng = nc.scalar if tokblk >= 28 else nc.sync
                eng.dma_start(y[bass.ts(tokblk, 128), :], ysb[:])
        return (1024, emit)

    # ---------- main loop ----------
    # blocks in emission order; each block is 4 (c=0) or 8 score pairs
    blocks = [(c, th, h) for c in range(C) for th in range(2) for h in range(HPC)]
    total_pairs = sum(4 if c == 0 else 8 for (c, th, h) in blocks)

    # chunk 0 projections: Q/K for j=0,1 and V tb=0..7
    for j in (0, 1):
        for proj, dst, nm in ((1, kt_sb, "k"), (0, qt_sb, "q")):
            for term in range(3):
                push(*proj_qk_group(j, proj, dst, term, nm))
    for tb in range(8):
        push(*proj_v(tb))

    # drain K j0 + Q j0 inline so the first quad can start immediately
    drain_cycles(6 * 1024)

    # total deferred PE cycles: projections for all 4 chunks + AV + finish
    total_filler = (4 * (12 * 1024 + 8 * 768)        # projections
                    + 4 * 4 * 520 + 12 * 4 * 1040    # AV (c0 / c1-3)
                    + 32 * 128 + 32 * 1024)          # fin1 + fin2
    per_pair = -(-total_filler // total_pairs)

    pairs_done = 0
    carry = 0
    for bi, (c, th, h) in enumerate(blocks):
        cts = list(range(max(0, 8 * (c - 1)), 8 * (c + 1)))
        pairs = sorted({ct // 2 for ct in cts})
        tok = bass.ds(c * L + th * 512, 512)

        # enqueue next chunk's projections at the start of each chunk
        if th == 0 and h == 1 and c + 1 < C:
            for j in (2 * (c + 1), 2 * (c + 1) + 1):
                for proj, dst, nm in ((1, kt_sb, "k"), (0, qt_sb, "q")):
                    for term in range(3):
                        push(*proj_qk_group(j, proj, dst, term, nm))
            for tb in range(8 * (c + 1), 8 * (c + 2)):
                push(*proj_v(tb))

        for p in pairs:
            sc = ps.tile([128, 1024], F32, tag="sc", bufs=2,
                         name=f"sc{bi}_{p}")
            for half in range(2):
                ct = 2 * p + half
                nc.tensor.matmul(
                    sc[:, bass.ts(half, 512)],
                    kt_sb[h * HD:(h + 1) * HD, bass.ts(ct, 128)],
                    qt_sb[h * HD:(h + 1) * HD, tok],
                    start=True, stop=True)
            at = a_pool.tile([128, 1024], BF16, tag="a", name=f"a{bi}_{p}")
            nc.scalar.activation(at[:], sc[:],
                                 mybir.ActivationFunctionType.Exp,
                                 scale=EXP_SCALE)
            a_tiles[(c, th, h, p)] = at
            pairs_done += 1
            remaining = total_pairs - pairs_done
            if remaining > 0:
                if c == C - 1:
                    # last chunk brings no projection fillers: spread what's
                    # queued evenly over the remaining pairs
                    budget = max(256, filler_cycles // remaining)
                    drain_cycles(budget)
                else:
                    # cap each inter-pair drain so the Act engine's exp
                    # stream never starves during chunk-start proj surges
                    budget = min(per_pair + carry, 2304)
                    spent = drain_cycles(budget)
                    carry = min(per_pair + carry - spent, 4096)

        # deferred: this block's AV matmuls and normalize, staggered so each
        # DVE normalize trails its AV by >=1 filler and each consumer of a
        # cross-engine product (transpose<-norm, outproj<-ot copy) trails by
        # >=2 fillers of PE work.
        last = h == HPC - 1
        units = []
        for tb in range(4):
            units.append(av_head(c, th, h, tb, cts))
            if tb >= 1:
                units.append(norm(c, th, h, tb - 1))
            if last and tb >= 2:
                units.append(fin1(c * 8 + th * 4 + tb - 2))
            if last and tb >= 3:
                units.append(fin2(c * 8 + th * 4 + tb - 3))
        units.append(norm(c, th, h, 3))
        if last:
            tb0 = c * 8 + th * 4
            units.append(fin1(tb0 + 2))
            units.append(fin2(tb0 + 1))
            units.append(fin1(tb0 + 3))
            units.append(fin2(tb0 + 2))
            units.append(fin2(tb0 + 3))
        for u in units:
            push(*u)

    drain_all()


_CACHE = {}


def _build():
    if "nc" in _CACHE:
        return _CACHE["nc"]
    nc = bacc.Bacc(
        "TRN2",
        target_bir_lowering=False,
        debug=False,
        enable_asserts=False,
        num_devices=NCORES,
    )
    ins = {
        "wpk": nc.dram_tensor("wpk", [128, NW, 2, HDB], FP8,
                              kind="ExternalInput").ap(),
        "wo": nc.dram_tensor("wo", [HDB, D], BF16, kind="ExternalInput").ap(),
        "ident": nc.dram_tensor("ident", [128, 128], BF16,
                                kind="ExternalInput").ap(),
    }
    ins["x8"] = nc.dram_tensor("x8", [128, NDR, 2, T], FP8,
                               kind="ExternalInput").ap()
    ins["x8lo"] = nc.dram_tensor("x8lo", [128, NDR, 2, T], FP8,
                                 kind="ExternalInput").ap()
    outs = {"y": nc.dram_tensor("y", [T, D], BF16, kind="ExternalOutput").ap()}
    with tile.TileContext(nc, trace_sim=False) as tc:
        with ExitStack() as kctx:
            build_kernel(nc, tc, outs, ins, kctx)
    nc.compile()
    _CACHE["nc"] = nc
    return nc


def make_in_maps(x, Wq, Wk, Wv, Wo, bo):
    f8 = ml_dtypes.float8_e4m3
    xt32 = np.asarray(x, dtype=np.float32).reshape(T, D).T  # [D, T]
    # DoubleRow layout [r][p, i, t] = X^T[r*256 + i*128 + p, t], hi + residual
    xdr = np.ascontiguousarray(xt32.reshape(NDR, 2, 128, T).transpose(0, 2, 1, 3))
    x8 = xdr.astype(f8)
    x8lo = (xdr - x8.astype(np.float32)).astype(f8)
    x8t = np.ascontiguousarray(x8.transpose(1, 0, 2, 3))      # [128, NDR, 2, T]
    x8lot = np.ascontiguousarray(x8lo.transpose(1, 0, 2, 3))
    ident = np.eye(128, dtype=np.float32).astype(ml_dtypes.bfloat16)

    def wdr(w, hs, scale):
        """[D, 128] weight slice -> DR layout [NDR, 128, 2, 128] hi + lo."""
        m = np.asarray(w, np.float32)[:, hs] * scale
        m = np.ascontiguousarray(m.reshape(NDR, 2, 128, HDB).transpose(0, 2, 1, 3))
        hi = m.astype(f8)
        lo = (m - hi.astype(np.float32)).astype(f8)
        return hi, lo

    in_maps = []
    for core in range(NCORES):
        hs = slice(core * HDB, (core + 1) * HDB)
        wo = np.ascontiguousarray(
            np.asarray(Wo, np.float32)[hs, :]).astype(ml_dtypes.bfloat16)
        # packed weight tensor: [K-hi, Q-hi, K-lo, Q-lo, V-hi, V-lo] x NDR
        # of [128, 2, 128] (matches WIDX in build_kernel)
        qhi, qlo = wdr(Wq, hs, QKSCALE)
        khi, klo = wdr(Wk, hs, QKSCALE)
        vhi, vlo = wdr(Wv, hs, VSCALE)
        blocks = [khi, qhi, klo, qlo, vhi, vlo]
        wpk = np.ascontiguousarray(
            np.stack(blocks, axis=0)            # [6, NDR, 128, 2, 128]
            .transpose(2, 0, 1, 3, 4)           # [128, 6, NDR, 2, 128]
            .reshape(128, NW, 2, HDB))
        im = {"wpk": wpk, "wo": wo, "ident": ident,
              "x8": x8t, "x8lo": x8lot}
        in_maps.append(im)
    return in_maps


def kernel(x, Wq, Wk, Wv, Wo, bo, _trace=False, _tmpdir=None):
    nc = _build()
    in_maps = make_in_maps(x, Wq, Wk, Wv, Wo, bo)
    res = run_bass_kernel_spmd(
        nc, in_maps, core_ids=list(range(NCORES)),
        trace=_trace, tmpdir=_tmpdir,
        **({"trace_cores": list(range(NCORES))} if _trace else {}),
    )
    if _trace:
        kernel.last_results = res
    y = np.zeros((T, D), dtype=np.float32)
    for r in res.results:
        y += np.asarray(r["y"], dtype=np.float32)
    y += np.asarray(bo, dtype=np.float32).reshape(1, D)
    return y.reshape(B, F, N, D)
